# revision 21
# baseline (speedup 1.0000x reference)
"""Trainium2 Bass kernel for nn_BaselineNeuralODE (v3).

Sharding (per spec hint): pure data parallelism over the num_features
axis (512 features -> 64 per core on 8 cores), replicated weights, no
collectives.  Activations are laid out transposed on chip: [channel on
partitions, feature on the free axis]; all matmuls are weights-stationary
bf16 (lhsT = 128x128 weight block, rhs = [128,64] activation slice).

RK4 3/8 in u-space (u = y@W1, W21 = W2@W1 fused in f64, dt baked into
the bf16 weights) with PSUM-continuation stages: each stage's matmul
group accumulates (start=False) onto a psum bank preloaded by a DVE
write, so the inter-stage critical path is just mm-group -> tanh:
  bankA (holds u1)   += (dt/3) a1@W21   -> u2   [in-place on prev bankD]
  bankB (preload c1) +=  dt    a2@W21   -> u3,  c1 = 2u1 - u2
  bankC (preload c3) +=  dt    a3@W21   -> u4,  c3 = 2u2 - u3
  bankD (preload c6) += (dt/8) a4@W21   -> u1', c6 = .375u4+.75u3-.125u1
(the accumulate-onto-DVE-written-psum mechanism is validated on HW in
test_psumwrite.py; has_written bits persist across stopped groups, so
banks are warmed once at startup).

Encoder per step additionally: h_ode = h + S@((dt/8)W2e) accumulated
onto bank E preloaded with h; the GRU gate psum group collects
x@wi (rank-1 matmuls) + h@wh (early, h known at step start) +
S@((dt/8)W2e@wh) (fused, skips the h_ode->bf16 cast on the path);
blend h' = z*h_ode + (1-z)*n with z-products computed off-path and h'
written twice (bf16 copy feeds next step's matmuls, f32 is the state).

Decoder (fused, streaming): r_i = r_{i-1} + S_i @ ((dt/8) W2d@D1),
pred_i = tanh(r_i)@D2, deferred by one step into engine idle gaps;
one output DMA at the end.

All matmuls bf16 (error ~6e-3 vs the 2e-2 budget); all state f32.
"""

import numpy as np
from contextlib import ExitStack

import concourse.bass as bass
import concourse.tile as tile
from concourse import mybir
from concourse.bass_utils import run_bass_kernel_spmd

AF = mybir.ActivationFunctionType
OP = mybir.AluOpType
F32 = mybir.dt.float32
BF16 = mybir.dt.bfloat16

TC, TT = 128, 256
F, L = 512, 256
NCORES = 8
FL = F // NCORES            # 64 features per core
NE = TC                     # encoder steps
NL = TT - 1                 # latent steps
TRACE = False

_cache = {}

# weight name -> (nk, nj) 128x128 blocking of the [in, out] matrix
WSPECS = {
    "W1e": (2, 4), "W21e3": (4, 4), "W21e1": (4, 4), "W2e8": (4, 2),
    "wh": (2, 6), "W2wh": (4, 6),
    "W1d": (2, 4), "W21d3": (4, 4), "W21d1": (4, 4), "W21d8": (4, 4),
    "W2D1s": (4, 2), "D1": (2, 2),
}


def _split_waits(nc):
    """Walrus allows only 1 inline sync-wait per instruction; Tile can attach
    more. Move excess waits onto same-engine InstNoOp's inserted just before
    the instruction (engine streams are extracted in block order)."""
    nop_id = [0]
    for f in nc.m.functions:
        for bb in f.blocks:
            insts = list(bb.instructions)
            out = []
            changed = False
            for inst in insts:
                si = inst.sync_info
                waits = list(si.on_wait) if si is not None and si.on_wait else []
                if len(waits) > 1:
                    for w in waits[:-1]:
                        nop_id[0] += 1
                        out.append(mybir.InstNoOp(
                            name=f"I-waitnop-{nop_id[0]}", ins=[], outs=[],
                            engine=inst.engine,
                            sync_info=mybir.SyncInfo(on_wait=[w], on_update=[])))
                    inst.sync_info = mybir.SyncInfo(on_wait=waits[-1:],
                                                    on_update=list(si.on_update))
                    changed = True
                out.append(inst)
            if changed:
                bb.instructions = out


def _block_w(W, nk, nj):
    """[K, M] -> [128, nk*nj*128]; block (k, j) at cols ((k*nj)+j)*128."""
    K, M = W.shape
    assert K == nk * 128 and M == nj * 128, (W.shape, nk, nj)
    return np.ascontiguousarray(
        W.reshape(nk, 128, nj, 128).transpose(1, 0, 2, 3).reshape(128, nk * nj * 128))


def _bf(x):
    import ml_dtypes
    return np.asarray(x, ml_dtypes.bfloat16)


class _Builder:
    def build(self, split_waits=True):
        nc = bass.Bass("TRN2", target_bir_lowering=False, debug=False)
        self.nc = nc
        dram = {}
        for name, (nk, nj) in WSPECS.items():
            dram[name] = nc.dram_tensor(name, [128, nk * nj * 128], BF16,
                                        kind="ExternalInput").ap()
        dram["D2"] = nc.dram_tensor("D2", [128, 2], BF16,
                                    kind="ExternalInput").ap()
        dram["wi"] = nc.dram_tensor("wi", [128, 6], F32,
                                    kind="ExternalInput").ap()
        dram["wi1"] = nc.dram_tensor("wi1", [1, 768], BF16,
                                     kind="ExternalInput").ap()
        dram["xs1"] = nc.dram_tensor("xs1", [1, NE * FL], BF16,
                                     kind="ExternalInput").ap()
        dram["cv_rev"] = nc.dram_tensor("cv_rev", [NE * FL], F32,
                                        kind="ExternalInput").ap()
        out_dram = nc.dram_tensor("out", [1, (NL + 1) * FL], F32,
                                  kind="ExternalOutput").ap()
        self.dram = dram

        with tile.TileContext(nc) as tc:
            with ExitStack() as ctx:
                self._body(ctx, tc, out_dram)
        if split_waits:
            _split_waits(nc)
        return nc

    # -- matmul emission ----------------------------------------------------
    def mm_group(self, ps, wname, rhs, start=True, stop=True, skip=False):
        """ps[:, j*64:(j+1)*64] (+)= sum_k W[k,j].T @ rhs[:, k*64:(k+1)*64]."""
        nc = self.nc
        nk, nj = WSPECS[wname]
        w = self.wsb[wname]
        ops = []
        for j in range(nj):
            for k in range(nk):
                ops.append((w[:, ((k * nj) + j) * 128:((k * nj) + j + 1) * 128],
                            rhs[:, k * 64:(k + 1) * 64],
                            (j * 64, (j + 1) * 64)))
        n = len(ops)
        for i, (wap, rap, sl) in enumerate(ops):
            nc.tensor.matmul(ps[:, sl[0]:sl[1]], lhsT=wap, rhs=rap,
                             start=(i == 0 and start),
                             stop=(i == n - 1 and stop),
                             skip_group_check=skip)

    def stage_mms(self, ps, wname, rhs):
        """Continuation group: accumulate onto carried/DVE-preloaded psum."""
        self.mm_group(ps, wname, rhs, start=False, stop=True, skip=True)

    def stage_mms_h(self, ps, wname, rhs):
        """Continuation group in half-blocks: (j01,k01), (j01,k23), (j23,*).
        Tile tracks deps per byte range, so the first 4 matmuls only wait on
        the LEFT halves of the tanh output and the preload write - they can
        start while the right halves are still being produced."""
        nc = self.nc
        nk, nj = WSPECS[wname]
        assert (nk, nj) == (4, 4)
        w = self.wsb[wname]
        order = []
        for jh in range(2):
            for kh in range(2):
                for j in (2 * jh, 2 * jh + 1):
                    for k in (2 * kh, 2 * kh + 1):
                        order.append((j, k))
        n = len(order)
        for idx, (j, k) in enumerate(order):
            nc.tensor.matmul(ps[:, j * 64:(j + 1) * 64],
                             lhsT=w[:, ((k * nj) + j) * 128:
                                    ((k * nj) + j + 1) * 128],
                             rhs=rhs[:, k * 64:(k + 1) * 64],
                             start=False, stop=(idx == n - 1),
                             skip_group_check=True)

    def tanh_h(self, src, tag):
        """tanh in two half ops (left half usable before right is done)."""
        nc = self.nc
        a = self.pool.tile([128, 256], BF16, tag=tag, name=f"a_{tag}")
        nc.scalar.activation(a[:, 0:128], src[:, 0:128], AF.Tanh)
        nc.scalar.activation(a[:, 128:256], src[:, 128:256], AF.Tanh)
        return a

    def copy_h(self, src, tag, dt=F32):
        """psum -> sbuf copy in two half ops."""
        nc = self.nc
        t = self.pool.tile([128, 256], dt, tag=tag, name=f"c_{tag}")
        nc.vector.tensor_copy(t[:, 0:128], src[:, 0:128])
        nc.vector.tensor_copy(t[:, 128:256], src[:, 128:256])
        return t

    def stt_h(self, out, in0, scalar, in1, op0, op1):
        """scalar_tensor_tensor in two half ops (for psum preload writes)."""
        nc = self.nc
        for c0, c1 in ((0, 128), (128, 256)):
            nc.vector.scalar_tensor_tensor(out[:, c0:c1], in0[:, c0:c1],
                                           scalar, in1[:, c0:c1], op0, op1)

    # -- GRU pieces ---------------------------------------------------------
    def gru_pre(self, s):
        """Open the gh psum group with rank-1 x@wi preloads (r/z blocks)."""
        nc = self.nc
        ghps = self.psum.tile([128, 384], F32, tag="G", padded_shape=[128, 512])
        for j in range(4):
            nc.tensor.matmul(ghps[:, j * 64:(j + 1) * 64],
                             lhsT=self.wi1[0:1, j * 128:(j + 1) * 128],
                             rhs=self.xs1[0:1, s * FL:(s + 1) * FL],
                             start=(j == 0), stop=False)
        return ghps

    def gru_hwh(self, ghps, hb, stop=False):
        """h@wh into the gh group (h known at step start -> emitted early)."""
        nc = self.nc
        wh = self.wsb["wh"]
        ops = []
        for j in range(6):
            for k in range(2):
                ops.append((wh[:, ((k * 6) + j) * 128:((k * 6) + j + 1) * 128],
                            hb[:, k * 64:(k + 1) * 64], (j * 64, (j + 1) * 64)))
        for i, (wap, rap, sl) in enumerate(ops):
            nc.tensor.matmul(ghps[:, sl[0]:sl[1]], lhsT=wap, rhs=rap,
                             start=False, stop=(stop and i == len(ops) - 1))

    def gru_w2wh(self, ghps, S):
        """S@((dt/8) W2e@wh) closes the gh group (fused h_ode@wh path)."""
        nc = self.nc
        w = self.wsb["W2wh"]
        ops = []
        for j in range(6):
            for k in range(4):
                ops.append((w[:, ((k * 6) + j) * 128:((k * 6) + j + 1) * 128],
                            S[:, k * 64:(k + 1) * 64], (j * 64, (j + 1) * 64)))
        for i, (wap, rap, sl) in enumerate(ops):
            nc.tensor.matmul(ghps[:, sl[0]:sl[1]], lhsT=wap, rhs=rap,
                             start=False, stop=(i == len(ops) - 1))

    def xwi_n(self, s):
        """x * wi for the n gate, on the scalar engine (AP scale)."""
        nc = self.nc
        xw = self.pool.tile([128, 128], F32, tag="xwn")
        xs = self.xb[:, s, :]
        nc.scalar.activation(xw[:, 0:64], xs, AF.Copy, scale=self.wi[:, 4:5])
        nc.scalar.activation(xw[:, 64:128], xs, AF.Copy, scale=self.wi[:, 5:6])
        return xw

    def gru_tail(self, ghps, h_ode, xw):
        """sigmoid gates from psum, n gate, blend h' = z*h_ode + (1-z)*n.
        h_ode: psum AP (bank E) or a zero tile. Updates self.h (f32) and
        returns the bf16 copy of h'."""
        nc = self.nc
        pool = self.pool
        sig_r = pool.tile([128, 128], F32, tag="sgr")
        nc.scalar.activation(sig_r, ghps[:, 0:128], AF.Sigmoid)
        sig_z = pool.tile([128, 128], F32, tag="sgz")
        nc.scalar.activation(sig_z, ghps[:, 128:256], AF.Sigmoid)
        npre = pool.tile([128, 128], F32, tag="np")
        nc.vector.tensor_mul(npre, sig_r, ghps[:, 256:384])
        nc.vector.tensor_add(npre, npre, xw)
        n_sb = pool.tile([128, 128], F32, tag="n")
        nc.scalar.activation(n_sb, npre, AF.Tanh)
        q1 = pool.tile([128, 128], F32, tag="q1")
        nc.vector.tensor_mul(q1, sig_z, h_ode)
        om = pool.tile([128, 128], F32, tag="om")
        nc.vector.tensor_scalar(om, sig_z, -1.0, 1.0, OP.mult, OP.add)
        nm = pool.tile([128, 128], F32, tag="nm")
        nc.vector.tensor_mul(nm, om, n_sb)
        h_b = pool.tile([128, 128], BF16, tag="hb")
        nc.vector.tensor_add(h_b, q1, nm)
        nc.vector.tensor_add(self.h, q1, nm)
        return h_b

    # -- kernel body --------------------------------------------------------
    def _body(self, ctx, tc, out_dram):
        nc = self.nc
        singles = ctx.enter_context(tc.tile_pool(name="singles", bufs=1))
        state = ctx.enter_context(tc.tile_pool(name="state", bufs=1))
        pool = ctx.enter_context(tc.tile_pool(name="work", bufs=3))
        psum = ctx.enter_context(tc.tile_pool(name="psum", bufs=1, space="PSUM"))
        self.pool, self.psum = pool, psum

        # ---- weights / inputs ----
        self.wsb = {}
        for nm, (nk, nj) in WSPECS.items():
            t = singles.tile([128, nk * nj * 128], BF16, tag=f"w_{nm}")
            nc.sync.dma_start(out=t, in_=self.dram[nm])
            self.wsb[nm] = t
        d2 = singles.tile([128, 2], BF16, tag="w_D2")
        nc.sync.dma_start(out=d2, in_=self.dram["D2"])
        wi = singles.tile([128, 6], F32, tag="w_wi")
        nc.sync.dma_start(out=wi, in_=self.dram["wi"])
        wi1 = singles.tile([1, 768], BF16, tag="w_wi1")
        nc.sync.dma_start(out=wi1, in_=self.dram["wi1"])
        xs1 = singles.tile([1, NE * FL], BF16, tag="xs1")
        nc.sync.dma_start(out=xs1, in_=self.dram["xs1"])
        xb = singles.tile([128, NE, FL], F32, tag="xb")
        cv = self.dram["cv_rev"]
        bcast = bass.AP(tensor=cv.tensor, offset=cv.offset,
                        ap=[[0, 128]] + list(cv.ap))
        nc.gpsimd.dma_start(out=xb.rearrange("p t f -> p (t f)"), in_=bcast)
        self.wi, self.wi1, self.xs1, self.xb = wi, wi1, xs1, xb

        preds = singles.tile([1, (NL + 1) * FL], F32, tag="preds")

        h = state.tile([128, 128], F32, tag="h")
        nc.vector.memset(h, 0.0)
        zero_f = state.tile([128, 128], F32, tag="zf")
        nc.vector.memset(zero_f, 0.0)
        zero_b = state.tile([128, 128], BF16, tag="zb")
        nc.vector.memset(zero_b, 0.0)
        self.h = h

        # ---- persistent psum banks ----
        P = [psum.tile([128, 256], F32, tag=f"P{k}", name=f"P{k}",
                       padded_shape=[128, 512])
             for k in range(4)]
        E = psum.tile([128, 128], F32, tag="E", padded_shape=[128, 512])
        # warm has_written bits of banks whose first real group is start=False
        wz = self.wsb["W21d1"]
        for bank in (P[1], P[2]):
            nc.tensor.matmul(bank, lhsT=wz[:, 0:128], rhs=wz[:, 0:256],
                             start=True, stop=True)
        nc.tensor.matmul(E, lhsT=wz[:, 0:128], rhs=wz[:, 0:128],
                         start=True, stop=True)

        # ================= encoder =================
        h_b = zero_b
        for s in range(NE):
            xw = self.xwi_n(s)
            if s == 0:
                ghps = self.gru_pre(s)
                self.gru_hwh(ghps, h_b, stop=True)
                h_b = self.gru_tail(ghps, zero_f, xw)
                continue
            # preload bank E with h (h_ode accumulates on top)
            nc.vector.tensor_copy(E, h)
            # W1e + stage A: one accumulation group on P0
            self.mm_group(P[0], "W1e", h_b, start=True, stop=False)
            ghps = self.gru_pre(s)          # rank-1 mms fill the a1 gap
            a1 = self.tanh_h(P[0], "a1")
            u1sb = self.copy_h(P[0], "u1sb")
            self.stage_mms_h(P[0], "W21e3", a1)          # P0 = u2
            a2 = self.tanh_h(P[0], "a2")
            self.stt_h(P[1], u1sb, 2.0, P[0],
                       OP.mult, OP.subtract)             # c1 -> P1
            self.stage_mms_h(P[1], "W21e1", a2)          # P1 = u3
            a3 = self.tanh_h(P[1], "a3")
            u2sb = pool.tile([128, 256], F32, tag="u2sb")
            nc.vector.tensor_copy(u2sb, P[0])
            self.stt_h(P[2], u2sb, 2.0, P[1],
                       OP.mult, OP.subtract)             # c3 -> P2
            sp = pool.tile([128, 256], F32, tag="sp")
            nc.gpsimd.tensor_add(sp, a2, a3)
            sq = pool.tile([128, 256], F32, tag="sq")
            nc.gpsimd.tensor_add(sq, sp, sp)
            sr = pool.tile([128, 256], F32, tag="sr")
            nc.gpsimd.tensor_add(sr, sq, sp)
            sA = pool.tile([128, 256], F32, tag="sA")
            nc.gpsimd.tensor_add(sA, sr, a1)
            self.stage_mms_h(P[2], "W21e1", a3)          # P2 = u4
            self.gru_hwh(ghps, h_b)         # h@wh fills the S gap on PE
            a4 = self.tanh_h(P[2], "a4")
            S = pool.tile([128, 256], BF16, tag="S")
            nc.vector.tensor_add(S, sA, a4)
            self.gru_w2wh(ghps, S)                       # closes gh group
            self.stage_mms(E, "W2e8", S)                 # E = h + (dt/8) S@W2e
            h_b = self.gru_tail(ghps, E, xw)

        # ================= latent init =================
        self.mm_group(P[3], "W1d", h_b)                  # P3 = u1 = z0@W1d
        self.mm_group(E, "D1", h_b)                      # E = z0@D1
        r_acc = state.tile([128, 128], F32, tag="racc")
        nc.vector.tensor_copy(r_acc, E)

        # deferred decode tail: emitted after the NEXT step's stage-A head so
        # its DVE/ACT/PE ops land in idle slots instead of gating the head
        pending = [0]
        pend_cp = [None]

        def decode_flush():
            if pend_cp[0] is not None:
                j, pps_old = pend_cp[0]
                nc.scalar.copy(preds[0:1, j * FL:(j + 1) * FL],
                               pps_old[0:1, 0:FL])
                pend_cp[0] = None
            if pending[0] is None:
                return
            i = pending[0]
            nc.vector.tensor_add(r_acc, E, r_acc)
            rt = pool.tile([128, 128], BF16, tag="rt")
            nc.scalar.activation(rt, r_acc, AF.Tanh)
            pps = psum.tile([1, FL], F32, tag="FF", padded_shape=[128, 512])
            for k in range(2):
                nc.tensor.matmul(pps[0:1, 0:FL], lhsT=d2[:, k:k + 1],
                                 rhs=rt[:, k * 64:(k + 1) * 64],
                                 start=(k == 0), stop=(k == 1))
            pend_cp[0] = (i, pps)
            pending[0] = None

        # prediction for t0 (z0) before any step; r_acc add is a no-op so
        # emit rt/p directly
        rt0 = pool.tile([128, 128], BF16, tag="rt")
        nc.scalar.activation(rt0, r_acc, AF.Tanh)
        pps0 = psum.tile([1, FL], F32, tag="FF", padded_shape=[128, 512])
        for k in range(2):
            nc.tensor.matmul(pps0[0:1, 0:FL], lhsT=d2[:, k:k + 1],
                             rhs=rt0[:, k * 64:(k + 1) * 64],
                             start=(k == 0), stop=(k == 1))
        pend_cp[0] = (0, pps0)
        pending[0] = None

        # ================= latent steps =================
        for i in range(1, NL + 1):
            bA = P[(3 * i) % 4]
            bB = P[(3 * i + 1) % 4]
            bC = P[(3 * i + 2) % 4]
            bD = P[(3 * i + 3) % 4]
            # -- step head: emitted before the previous step's decode tail --
            a1 = self.tanh_h(bA, "a1")
            u1sb = self.copy_h(bA, "u1sb")
            self.stage_mms_h(bA, "W21d3", a1)            # bA = u2
            decode_flush()                               # tail of step i-1
            u1_8 = pool.tile([128, 256], F32, tag="u18")
            nc.vector.tensor_scalar_mul(u1_8, u1sb, 0.125)
            a2 = self.tanh_h(bA, "a2")
            self.stt_h(bB, u1sb, 2.0, bA,
                       OP.mult, OP.subtract)             # c1 -> bB
            self.stage_mms_h(bB, "W21d1", a2)            # bB = u3
            a3 = self.tanh_h(bB, "a3")
            u2sb = pool.tile([128, 256], F32, tag="u2sb")
            nc.vector.tensor_copy(u2sb, bA)
            self.stt_h(bC, u2sb, 2.0, bB,
                       OP.mult, OP.subtract)             # c3 -> bC
            qp = pool.tile([128, 256], F32, tag="qp")
            nc.vector.scalar_tensor_tensor(qp, bB, 0.75, u1_8,
                                           OP.mult, OP.subtract)
            sp = pool.tile([128, 256], F32, tag="sp")
            nc.gpsimd.tensor_add(sp, a2, a3)
            sq = pool.tile([128, 256], F32, tag="sq")
            nc.gpsimd.tensor_add(sq, sp, sp)
            sr = pool.tile([128, 256], F32, tag="sr")
            nc.gpsimd.tensor_add(sr, sq, sp)
            self.stage_mms_h(bC, "W21d1", a3)            # bC = u4
            a4 = self.tanh_h(bC, "a4")
            self.stt_h(bD, bC, 0.375, qp,
                       OP.mult, OP.add)                  # c6 -> bD
            sA = pool.tile([128, 256], F32, tag="sA")
            nc.gpsimd.tensor_add(sA, sr, a1)
            S = pool.tile([128, 256], BF16, tag="S")
            nc.vector.tensor_add(S, sA, a4)
            self.stage_mms_h(bD, "W21d8", a4)            # bD = u1'
            self.mm_group(E, "W2D1s", S)                 # dr
            pending[0] = i
        decode_flush()
        if pend_cp[0] is not None:
            j, pps_old = pend_cp[0]
            nc.scalar.copy(preds[0:1, j * FL:(j + 1) * FL],
                           pps_old[0:1, 0:FL])

        nc.sync.dma_start(out=out_dram, in_=preds)


def _prepare(inputs):
    ct = np.asarray(inputs["context_times"], np.float64)
    tt = np.asarray(inputs["target_times"], np.float64)
    rev_t = ct[::-1]
    dts_enc = rev_t[:-1] - rev_t[1:]
    dts_lat = tt[1:] - tt[:-1]
    dt_e = float(np.mean(dts_enc))
    dt_l = float(np.mean(dts_lat))
    assert np.allclose(dts_enc, dt_e, rtol=1e-4), "encoder dt not constant"
    assert np.allclose(dts_lat, dt_l, rtol=1e-4), "latent dt not constant"
    assert dt_e > 0 and dt_l > 0

    for nm in ("enc_b1", "enc_b2", "gru_bi", "gru_bh", "dyn_b1", "dyn_b2",
               "dec_b1", "dec_b2"):
        assert not np.any(np.asarray(inputs[nm])), f"nonzero bias {nm}"
    assert np.all(np.asarray(inputs["context_mask"]) == 1.0), "mask must be 1"

    f64 = np.float64
    enc_w1 = np.asarray(inputs["enc_w1"], f64)
    enc_w2 = np.asarray(inputs["enc_w2"], f64)
    dyn_w1 = np.asarray(inputs["dyn_w1"], f64)
    dyn_w2 = np.asarray(inputs["dyn_w2"], f64)
    dec_w1 = np.asarray(inputs["dec_w1"], f64)
    dec_w2 = np.asarray(inputs["dec_w2"], np.float32)
    gru_wh = np.asarray(inputs["gru_wh"], f64)
    gru_wi = np.asarray(inputs["gru_wi"], np.float32)

    W21e = enc_w2 @ enc_w1
    W21d = dyn_w2 @ dyn_w1
    Ws = {
        "W1e": enc_w1,
        "W21e3": (dt_e / 3.0) * W21e,
        "W21e1": dt_e * W21e,
        "W2e8": (dt_e / 8.0) * enc_w2,
        "wh": gru_wh,
        "W2wh": (dt_e / 8.0) * (enc_w2 @ gru_wh),
        "W1d": dyn_w1,
        "W21d3": (dt_l / 3.0) * W21d,
        "W21d1": dt_l * W21d,
        "W21d8": (dt_l / 8.0) * W21d,
        "W2D1s": (dt_l / 8.0) * (dyn_w2 @ dec_w1),
        "D1": dec_w1,
    }
    wdata = {}
    for name, (nk, nj) in WSPECS.items():
        wdata[name] = _bf(_block_w(np.asarray(Ws[name], np.float32), nk, nj))
    wdata["D2"] = _bf(np.ascontiguousarray(dec_w2.reshape(2, 128).T))
    wdata["wi"] = np.ascontiguousarray(gru_wi.reshape(6, 128).T)
    wdata["wi1"] = _bf(gru_wi.reshape(1, 768))

    cv = np.asarray(inputs["context_values"], np.float32)
    rev_v = np.ascontiguousarray(cv[::-1])
    key = (round(dt_e, 9), round(dt_l, 9), "v3")
    return key, wdata, rev_v


def kernel(**inputs):
    key, wdata, rev_v = _prepare(inputs)
    if key not in _cache:
        _cache[key] = _Builder().build()
    nc = _cache[key]

    in_maps = []
    for c in range(NCORES):
        m = dict(wdata)
        shard = np.ascontiguousarray(rev_v[:, c * FL:(c + 1) * FL])
        m["cv_rev"] = shard.reshape(-1)
        m["xs1"] = _bf(shard.reshape(1, -1))
        in_maps.append(m)
    res = run_bass_kernel_spmd(nc, in_maps, core_ids=list(range(NCORES)),
                               trace=TRACE)
    kernel.last_results = res
    out = np.concatenate(
        [res.results[c]["out"].reshape(TT, FL) for c in range(NCORES)], axis=1)
    return out.astype(np.float32)


# revision 28
# speedup vs baseline: 1.1281x; 1.1281x over previous
"""Trainium2 Bass kernel for nn_BaselineNeuralODE (v2).

Sharding (per spec hint): pure data parallelism over the num_features
axis (512 features -> 64 per core on 8 cores), replicated weights, no
collectives.  Activations are laid out transposed on chip: [channel on
partitions, feature on the free axis], so every matmul is
weights-stationary (lhsT = 128x128 bf16 weight block, rhs = [128, 64]
activation slice) and no transposes are needed.

Math (validated against reference): f(y) = tanh(y@W1)@W2 with the RK4
3/8 rule is evaluated in "u-space" (u = y@W1, W21 = W2@W1, f64-fused):
  a_i = tanh(u_i), gt_i = dt * a_i@W21   (dt baked into bf16 weights)
  u2 = u1 + gt1/3;  u3 = c1 + gt2;  u4 = c3 + gt3
  u1' = c6 + gt4/8                       (latent recurrence)
  h'  = h + S@((dt/8) W2e), S = a1 + 3a2 + 3a3 + a4   (encoder)
with SBUF-only re-associations kept OFF the critical path on GpSimd:
  c1 = 2u1 - u2;  c3 = 2u2 - u3;  c6 = (6u3 + 3u4 - u1)/8
so the inter-stage critical path is: mm-group -> one DVE op -> tanh.

GRU: input gates x@wi are preloaded into the gh psum accumulation group
as rank-1 matmuls (lhsT = wi row block [1,128], rhs = x row [1,64]), so
sigmoid reads the psum directly; the n-gate input adds via two small
scalar_tensor_tensor ops split across DVE/GpSimd.

Decoder (fused, streaming): r_i = r_{i-1} + S_i @ ((dt/8) W2d@D1),
pred_i = tanh(r_i)@D2.  The per-step decode (8+2 matmuls, 1 DVE add,
1 tanh) is deferred by one step in the instruction streams so it fills
engine idle gaps; output staged in SBUF, one DMA at the end.

All matmuls bf16 (error ~6e-3 vs the 2e-2 budget); all state f32.
"""

import numpy as np
from contextlib import ExitStack

import concourse.bass as bass
import concourse.tile as tile
from concourse import mybir
from concourse.bass_utils import run_bass_kernel_spmd

AF = mybir.ActivationFunctionType
OP = mybir.AluOpType
F32 = mybir.dt.float32
BF16 = mybir.dt.bfloat16

TC, TT = 128, 256
F, L = 512, 256
NCORES = 8
FL = F // NCORES            # 64 features per core
NE = TC                     # encoder steps
NL = TT - 1                 # latent steps
TRACE = False

_cache = {}

# weight name -> (nk, nj) 128x128 blocking of the [in, out] matrix
WSPECS = {
    "W1e": (2, 4), "W21e1": (4, 4), "W2e8": (4, 2), "wh": (2, 6),
    "W1d": (2, 4), "W21d1": (4, 4), "W2D1s": (4, 2), "D1": (2, 2),
}


def _split_waits(nc):
    """Walrus allows only 1 inline sync-wait per instruction; Tile can attach
    more. Move excess waits onto same-engine InstNoOp's inserted just before
    the instruction (engine streams are extracted in block order)."""
    nop_id = [0]
    for f in nc.m.functions:
        for bb in f.blocks:
            insts = list(bb.instructions)
            out = []
            changed = False
            for inst in insts:
                si = inst.sync_info
                waits = list(si.on_wait) if si is not None and si.on_wait else []
                if len(waits) > 1:
                    for w in waits[:-1]:
                        nop_id[0] += 1
                        out.append(mybir.InstNoOp(
                            name=f"I-waitnop-{nop_id[0]}", ins=[], outs=[],
                            engine=inst.engine,
                            sync_info=mybir.SyncInfo(on_wait=[w], on_update=[])))
                    inst.sync_info = mybir.SyncInfo(on_wait=waits[-1:],
                                                    on_update=list(si.on_update))
                    changed = True
                out.append(inst)
            if changed:
                bb.instructions = out


def _block_w(W, nk, nj):
    """[K, M] -> [128, nk*nj*128]; block (k, j) at cols ((k*nj)+j)*128."""
    K, M = W.shape
    assert K == nk * 128 and M == nj * 128, (W.shape, nk, nj)
    return np.ascontiguousarray(
        W.reshape(nk, 128, nj, 128).transpose(1, 0, 2, 3).reshape(128, nk * nj * 128))


def _bf(x):
    import ml_dtypes
    return np.asarray(x, ml_dtypes.bfloat16)


class _Builder:
    def build(self, split_waits=True):
        nc = bass.Bass("TRN2", target_bir_lowering=False, debug=False)
        self.nc = nc
        dram = {}
        for name, (nk, nj) in WSPECS.items():
            dram[name] = nc.dram_tensor(name, [128, nk * nj * 128], BF16,
                                        kind="ExternalInput").ap()
        dram["D2"] = nc.dram_tensor("D2", [128, 2], BF16,
                                    kind="ExternalInput").ap()
        dram["wi"] = nc.dram_tensor("wi", [128, 6], F32,
                                    kind="ExternalInput").ap()
        dram["wi1"] = nc.dram_tensor("wi1", [1, 768], BF16,
                                     kind="ExternalInput").ap()
        dram["xs1"] = nc.dram_tensor("xs1", [1, NE * FL], BF16,
                                     kind="ExternalInput").ap()
        dram["cv_rev"] = nc.dram_tensor("cv_rev", [NE * FL], F32,
                                        kind="ExternalInput").ap()
        out_dram = nc.dram_tensor("out", [1, (NL + 1) * FL], F32,
                                  kind="ExternalOutput").ap()
        self.dram = dram

        with tile.TileContext(nc) as tc:
            with ExitStack() as ctx:
                self._body(ctx, tc, out_dram)
        if split_waits:
            _split_waits(nc)
        return nc

    def fill(self, n, tag):
        """PE warmers: dependency-free matmuls into a dead psum bank to keep
        the tensor engine busy through gaps (holds the HAM p-state at full
        clock). tag picks a bank unused in the current phase."""
        nc = self.nc
        fb = self.psum.tile([128, 256], F32, tag=tag, padded_shape=[128, 512])
        w = self.wsb["W21d1"]
        for _ in range(n):
            nc.tensor.matmul(fb, lhsT=w[:, 0:128], rhs=w[:, 0:256],
                             start=True, stop=True)

    def mm_group(self, ps, wname, rhs, pre_ops=None, start=True):
        """ps[:, j*64:(j+1)*64] (+)= sum_k W[k,j].T @ rhs[:, k*64:(k+1)*64].

        pre_ops: list of (lhsT_ap, rhs_ap, (c0, c1)) emitted first (same
        psum accumulation group)."""
        nc = self.nc
        nk, nj = WSPECS[wname]
        w = self.wsb[wname]
        ops = list(pre_ops) if pre_ops else []
        for j in range(nj):
            for k in range(nk):
                ops.append((w[:, ((k * nj) + j) * 128:((k * nj) + j + 1) * 128],
                            rhs[:, k * 64:(k + 1) * 64],
                            (j * 64, (j + 1) * 64)))
        n = len(ops)
        for i, (wap, rap, sl) in enumerate(ops):
            nc.tensor.matmul(ps[:, sl[0]:sl[1]], lhsT=wap, rhs=rap,
                             start=(i == 0 and start), stop=(i == n - 1))

    def rk4_stages(self, u1ps, u1, wname, want_c6, after_stage1=None,
                   fills=(0, 0, 0), fill_tag="A"):
        """Stages 1-3 of the 3/8 RK4 in g-space. u1ps: psum holding u1 (or
        None if u1 only in SBUF). Returns (a4, S, c6).

        Critical chain emitted first at every stage; SBUF-only combos go to
        GpSimd. after_stage1 (deferred decode tail) is emitted right after
        the g1 group so its PE/ACT ops fill the stage-2 dependency gap."""
        nc = self.nc
        pool, psum = self.pool, self.psum

        a1 = pool.tile([128, 256], BF16, tag="a1")
        nc.scalar.activation(a1, u1ps if u1ps is not None else u1, AF.Tanh)
        u1_8 = None
        if want_c6:
            u1_8 = pool.tile([128, 256], F32, tag="u18")
            nc.scalar.activation(u1_8, u1, AF.Copy, scale=0.125)
        g1 = psum.tile([128, 256], F32, tag="B", padded_shape=[128, 512])
        self.mm_group(g1, wname, a1)
        self.fill(fills[0], fill_tag)
        if after_stage1 is not None:
            after_stage1()

        u2 = pool.tile([128, 256], F32, tag="u2")
        nc.vector.scalar_tensor_tensor(u2, g1, 1.0 / 3.0, u1, OP.mult, OP.add)
        a2 = pool.tile([128, 256], BF16, tag="a2")
        nc.scalar.activation(a2, u2, AF.Tanh)
        c1 = pool.tile([128, 256], F32, tag="c1")
        nc.vector.scalar_tensor_tensor(c1, u1, 2.0, u2, OP.mult, OP.subtract)
        g2 = psum.tile([128, 256], F32, tag="C", padded_shape=[128, 512])
        self.mm_group(g2, wname, a2)
        self.fill(fills[1], fill_tag)

        u3 = pool.tile([128, 256], F32, tag="u3")
        nc.vector.tensor_add(u3, g2, c1)
        a3 = pool.tile([128, 256], BF16, tag="a3")
        nc.scalar.activation(a3, u3, AF.Tanh)
        c3 = pool.tile([128, 256], F32, tag="c3")
        nc.vector.scalar_tensor_tensor(c3, u2, 2.0, u3, OP.mult, OP.subtract)
        qp = None
        if want_c6:
            # c6 = 0.375 u4 + qp, qp = 0.75 u3 - u1/8; u1' = c6 + gt4/8
            qp = pool.tile([128, 256], F32, tag="qp")
            nc.vector.scalar_tensor_tensor(qp, u3, 0.75, u1_8,
                                           OP.mult, OP.subtract)
        # S = a1 + 3(a2 + a3) + a4 via scalar-free tensor_tensor on GpSimd
        sp = pool.tile([128, 256], F32, tag="sp")
        nc.gpsimd.tensor_add(sp, a2, a3)
        sq = pool.tile([128, 256], F32, tag="sq")
        nc.gpsimd.tensor_add(sq, sp, sp)
        sr = pool.tile([128, 256], F32, tag="sr")
        nc.gpsimd.tensor_add(sr, sq, sp)
        sA = pool.tile([128, 256], F32, tag="sA")
        nc.gpsimd.tensor_add(sA, sr, a1)
        g3 = psum.tile([128, 256], F32, tag="D", padded_shape=[128, 512])
        self.mm_group(g3, wname, a3)
        self.fill(fills[2], fill_tag)

        u4 = pool.tile([128, 256], F32, tag="u4")
        nc.vector.tensor_add(u4, g3, c3)
        a4 = pool.tile([128, 256], BF16, tag="a4")
        nc.scalar.activation(a4, u4, AF.Tanh)
        c6 = None
        if want_c6:
            c6 = pool.tile([128, 256], F32, tag="c6")
            nc.vector.scalar_tensor_tensor(c6, u4, 0.375, qp, OP.mult, OP.add)
        S = pool.tile([128, 256], BF16, tag="S")
        nc.vector.tensor_add(S, sA, a4)
        return a4, S, c6

    def xwi_n(self, s):
        """x * wi for the n gate, on the scalar engine (AP scale)."""
        nc = self.nc
        xw = self.pool.tile([128, 128], F32, tag="xwn")
        xs = self.xb[:, s, :]
        nc.scalar.activation(xw[:, 0:64], xs, AF.Copy, scale=self.wi[:, 4:5])
        nc.scalar.activation(xw[:, 64:128], xs, AF.Copy, scale=self.wi[:, 5:6])
        return xw

    def gru(self, s, ghps, hob, h_ode, xw):
        """GRU cell tail. ghps: [128,384] psum already holding x@wi in the
        r/z blocks (rank-1 preload mms emitted earlier). hob: bf16 h_ode.
        h_ode: f32 AP for the blend. xw: precomputed x*wi_n [128,128]."""
        nc = self.nc
        pool = self.pool
        wh = self.wsb["wh"]
        ops = []
        for j in range(6):
            for k in range(2):
                ops.append((wh[:, ((k * 6) + j) * 128:((k * 6) + j + 1) * 128],
                            hob[:, k * 64:(k + 1) * 64], (j * 64, (j + 1) * 64)))
        for i, (wap, rap, sl) in enumerate(ops):
            nc.tensor.matmul(ghps[:, sl[0]:sl[1]], lhsT=wap, rhs=rap,
                             start=False, stop=(i == len(ops) - 1))

        rz = pool.tile([128, 256], F32, tag="rz")
        nc.scalar.activation(rz, ghps[:, 0:256], AF.Sigmoid)
        npre = pool.tile([128, 128], F32, tag="np")
        nc.vector.tensor_mul(npre, rz[:, 0:128], ghps[:, 256:384])
        nc.vector.tensor_add(npre, npre, xw)
        n_sb = pool.tile([128, 128], F32, tag="n")
        nc.scalar.activation(n_sb, npre, AF.Tanh)
        t = pool.tile([128, 128], F32, tag="t")
        nc.vector.tensor_sub(t, h_ode, n_sb)
        t2 = pool.tile([128, 128], F32, tag="t2")
        nc.vector.tensor_mul(t2, rz[:, 128:256], t)
        nc.vector.tensor_add(self.h, t2, n_sb)

    def gru_pre(self, s):
        """Rank-1 x@wi preload mms opening the gh psum group (r/z blocks)."""
        nc = self.nc
        ghps = self.psum.tile([128, 384], F32, tag="G", padded_shape=[128, 512])
        xs1 = self.xs1
        for j in range(4):
            nc.tensor.matmul(ghps[:, j * 64:(j + 1) * 64],
                             lhsT=self.wi1[0:1, j * 128:(j + 1) * 128],
                             rhs=xs1[0:1, s * FL:(s + 1) * FL],
                             start=(j == 0), stop=False)
        return ghps

    def _body(self, ctx, tc, out_dram):
        nc = self.nc
        singles = ctx.enter_context(tc.tile_pool(name="singles", bufs=1))
        state = ctx.enter_context(tc.tile_pool(name="state", bufs=1))
        pool = ctx.enter_context(tc.tile_pool(name="work", bufs=3))
        psum = ctx.enter_context(tc.tile_pool(name="psum", bufs=1, space="PSUM"))
        self.pool, self.psum = pool, psum

        # ---- weights / inputs ----
        self.wsb = {}
        for nm, (nk, nj) in WSPECS.items():
            t = singles.tile([128, nk * nj * 128], BF16, tag=f"w_{nm}")
            nc.sync.dma_start(out=t, in_=self.dram[nm])
            self.wsb[nm] = t
        d2 = singles.tile([128, 2], BF16, tag="w_D2")
        nc.sync.dma_start(out=d2, in_=self.dram["D2"])
        wi = singles.tile([128, 6], F32, tag="w_wi")
        nc.sync.dma_start(out=wi, in_=self.dram["wi"])
        wi1 = singles.tile([1, 768], BF16, tag="w_wi1")
        nc.sync.dma_start(out=wi1, in_=self.dram["wi1"])
        xs1 = singles.tile([1, NE * FL], BF16, tag="xs1")
        nc.sync.dma_start(out=xs1, in_=self.dram["xs1"])
        xb = singles.tile([128, NE, FL], F32, tag="xb")
        cv = self.dram["cv_rev"]
        bcast = bass.AP(tensor=cv.tensor, offset=cv.offset,
                        ap=[[0, 128]] + list(cv.ap))
        nc.gpsimd.dma_start(out=xb.rearrange("p t f -> p (t f)"), in_=bcast)
        self.wi, self.wi1, self.xs1, self.xb = wi, wi1, xs1, xb

        preds = singles.tile([1, (NL + 1) * FL], F32, tag="preds")

        h = state.tile([128, 128], F32, tag="h")
        nc.vector.memset(h, 0.0)
        zero_f = state.tile([128, 128], F32, tag="zf")
        nc.vector.memset(zero_f, 0.0)
        zero_b = state.tile([128, 128], BF16, tag="zb")
        nc.vector.memset(zero_b, 0.0)
        self.h = h

        # ================= encoder =================
        for s in range(NE):
            xw = self.xwi_n(s)
            if s == 0:
                ghps = self.gru_pre(s)
                self.gru(s, ghps, zero_b, zero_f, xw)
                continue
            hb = pool.tile([128, 128], BF16, tag="hb")
            nc.scalar.activation(hb, h, AF.Copy)
            u1ps = psum.tile([128, 256], F32, tag="A", padded_shape=[128, 512])
            self.mm_group(u1ps, "W1e", hb)
            u1 = pool.tile([128, 256], F32, tag="u1e")
            nc.vector.tensor_copy(u1, u1ps)
            a4, S, _ = self.rk4_stages(u1ps, u1, "W21e1", want_c6=False,
                                       fills=(0, 0, 0), fill_tag="B2")
            # open the gh group early (PE fills the h_ode latency gap)
            ghps = self.gru_pre(s)
            Tps = psum.tile([128, 128], F32, tag="E", padded_shape=[128, 512])
            self.mm_group(Tps, "W2e8", S)
            h_ode = pool.tile([128, 128], F32, tag="hode")
            nc.vector.tensor_add(h_ode, Tps, h)
            hob = pool.tile([128, 128], BF16, tag="hob")
            nc.scalar.activation(hob, h_ode, AF.Copy)
            self.gru(s, ghps, hob, h_ode, xw)

        # ================= latent init =================
        zb = pool.tile([128, 128], BF16, tag="hb")
        nc.scalar.activation(zb, h, AF.Copy)
        u1ps = psum.tile([128, 256], F32, tag="A", padded_shape=[128, 512])
        self.mm_group(u1ps, "W1d", zb)
        u1 = state.tile([128, 256], F32, tag="u1")
        nc.vector.tensor_copy(u1, u1ps)
        r0ps = psum.tile([128, 128], F32, tag="E", padded_shape=[128, 512])
        self.mm_group(r0ps, "D1", zb)
        r_acc = state.tile([128, 128], F32, tag="racc")
        nc.vector.tensor_copy(r_acc, r0ps)

        # deferred decode tail: emitted one step later to fill idle gaps
        pending = [None]

        def decode_flush():
            if pending[0] is None:
                return
            i = pending[0]
            rt = pool.tile([128, 128], BF16, tag="rt")
            nc.scalar.activation(rt, r_acc, AF.Tanh)
            pps = psum.tile([1, FL], F32, tag="FF", padded_shape=[128, 512])
            for k in range(2):
                nc.tensor.matmul(pps[0:1, 0:FL], lhsT=d2[:, k:k + 1],
                                 rhs=rt[:, k * 64:(k + 1) * 64],
                                 start=(k == 0), stop=(k == 1))
            nc.scalar.copy(preds[0:1, i * FL:(i + 1) * FL], pps[0:1, 0:FL])
            pending[0] = None

        pending[0] = 0  # pred for t0 (z0)

        # ================= latent steps =================
        u1ps_cur = u1ps
        for i in range(1, NL + 1):
            a4, S, c6 = self.rk4_stages(u1ps_cur, u1, "W21d1", want_c6=True,
                                        after_stage1=decode_flush,
                                        fills=(0, 0, 0), fill_tag="A")
            u1ps_cur = None
            g4 = psum.tile([128, 256], F32, tag="B2", padded_shape=[128, 512])
            self.mm_group(g4, "W21d1", a4)
            nc.vector.scalar_tensor_tensor(u1, g4, 0.125, c6, OP.mult, OP.add)
            # decode accumulation for step i (matmuls fill the u1'->a1 gap;
            # tanh/D2/copy deferred into step i+1)
            drps = psum.tile([128, 128], F32, tag="E", padded_shape=[128, 512])
            self.mm_group(drps, "W2D1s", S)
            nc.vector.tensor_add(r_acc, drps, r_acc)
            pending[0] = i
        decode_flush()

        nc.sync.dma_start(out=out_dram, in_=preds)


def _prepare(inputs):
    ct = np.asarray(inputs["context_times"], np.float64)
    tt = np.asarray(inputs["target_times"], np.float64)
    rev_t = ct[::-1]
    dts_enc = rev_t[:-1] - rev_t[1:]          # dt for steps s=1..NE-1
    dts_lat = tt[1:] - tt[:-1]
    dt_e = float(np.mean(dts_enc))
    dt_l = float(np.mean(dts_lat))
    assert np.allclose(dts_enc, dt_e, rtol=1e-4), "encoder dt not constant"
    assert np.allclose(dts_lat, dt_l, rtol=1e-4), "latent dt not constant"
    assert dt_e > 0 and dt_l > 0

    for nm in ("enc_b1", "enc_b2", "gru_bi", "gru_bh", "dyn_b1", "dyn_b2",
               "dec_b1", "dec_b2"):
        assert not np.any(np.asarray(inputs[nm])), f"nonzero bias {nm}"
    assert np.all(np.asarray(inputs["context_mask"]) == 1.0), "mask must be 1"

    f64 = np.float64
    enc_w1 = np.asarray(inputs["enc_w1"], f64)
    enc_w2 = np.asarray(inputs["enc_w2"], f64)
    dyn_w1 = np.asarray(inputs["dyn_w1"], f64)
    dyn_w2 = np.asarray(inputs["dyn_w2"], f64)
    dec_w1 = np.asarray(inputs["dec_w1"], f64)
    dec_w2 = np.asarray(inputs["dec_w2"], np.float32)
    gru_wh = np.asarray(inputs["gru_wh"], f64)
    gru_wi = np.asarray(inputs["gru_wi"], np.float32)

    Ws = {
        "W1e": enc_w1,
        "W21e1": dt_e * (enc_w2 @ enc_w1),
        "W2e8": (dt_e / 8.0) * enc_w2,
        "wh": gru_wh,
        "W1d": dyn_w1,
        "W21d1": dt_l * (dyn_w2 @ dyn_w1),
        "W2D1s": (dt_l / 8.0) * (dyn_w2 @ dec_w1),
        "D1": dec_w1,
    }
    wdata = {}
    for name, (nk, nj) in WSPECS.items():
        wdata[name] = _bf(_block_w(np.asarray(Ws[name], np.float32), nk, nj))
    wdata["D2"] = _bf(np.ascontiguousarray(dec_w2.reshape(2, 128).T))
    wdata["wi"] = np.ascontiguousarray(gru_wi.reshape(6, 128).T)
    wdata["wi1"] = _bf(gru_wi.reshape(1, 768))

    cv = np.asarray(inputs["context_values"], np.float32)
    rev_v = np.ascontiguousarray(cv[::-1])
    key = (round(dt_e, 9), round(dt_l, 9), "v2")
    return key, wdata, rev_v


def kernel(**inputs):
    key, wdata, rev_v = _prepare(inputs)
    if key not in _cache:
        _cache[key] = _Builder().build()
    nc = _cache[key]

    in_maps = []
    for c in range(NCORES):
        m = dict(wdata)
        shard = np.ascontiguousarray(rev_v[:, c * FL:(c + 1) * FL])
        m["cv_rev"] = shard.reshape(-1)
        m["xs1"] = _bf(shard.reshape(1, -1))
        in_maps.append(m)
    res = run_bass_kernel_spmd(nc, in_maps, core_ids=list(range(NCORES)),
                               trace=TRACE)
    kernel.last_results = res
    out = np.concatenate(
        [res.results[c]["out"].reshape(TT, FL) for c in range(NCORES)], axis=1)
    return out.astype(np.float32)
